# revision 2
# baseline (speedup 1.0000x reference)
"""Multi-head attention (B=4, T=2048, D=2048, H=16, E=128) on 8 TRN2 NeuronCores.

Sharding: batch (4) x head-group (2 groups of 8 heads) -> 8 cores.
Per core: q/k/v projections for its 8 heads + softmax(QK^T/sqrt(E))V.

FP8 (e4m3) design:
  - all matmul operands are fp8e4; accumulation stays fp32 in PSUM.
  - projections + context + softmax-denominator use DoubleRow perf mode
    (two 128-deep contraction chunks per instruction, 0.5 cycles/row).
  - weights are host-prescaled by 64 so their sigma~1.4 avoids the fp8
    denormal range; the PSUM->SBUF copy rescales by 1/64 (DVE tensor_scalar).
  - scores S^T[k,q] = K Q^T per 128-key tile, plain fp8 matmul.
  - softmax: exp((s - 3*sqrt(E))/sqrt(E)) on ACT over [128,1024] tile pairs
    (the -3 shift cancels between numerator and denominator and keeps
    exp() outputs ~15 max, far below fp8e4's 240 ceiling).
  - denominator: DoubleRow matmul with an all-ones lhsT accumulating the
    fp8 P^T pair tiles over keys; replicated over all 128 partitions so the
    final normalize is a plain elementwise multiply.
  - output written as C^T [h,E,T] fp32; host transposes back to [h,T,E].

PE order is hand-interleaved: the next head's Q/K projection chains (and the
next quad's V chains) are issued between attention pair iterations so the
in-order PE queue fills the stalls where PE waits on ACT's exp.
"""

import math
import sys

sys.path.insert(0, "/opt/trn_rl_repo")

import ml_dtypes
import numpy as np

import concourse.bass as bass  # noqa: F401  (registers engine methods)
import concourse.mybir as mybir
import concourse.tile as tile
from concourse import bacc
from concourse.bass_utils import run_bass_kernel_spmd

B, T, D, H, E = 4, 2048, 2048, 16, 128
N_CORES = 8
H_LOC = H // 2          # heads per core
P = 128                 # partitions
DT = D // P             # contraction chunks for projections
DP = DT // 2            # DoubleRow pairs over D
KT = T // P             # key tiles
KP = KT // 2            # DoubleRow pairs over keys
QW = 512                # q-chunk width (one PSUM bank of fp32)
QC = T // QW
FP8 = mybir.dt.float8e4
F32 = mybir.dt.float32
DR = mybir.MatmulPerfMode.DoubleRow
EXP_SCALE = 1.0 / math.sqrt(E)
EXP_BIAS = -3.0         # softmax shift; cancels in the normalize
WS = 64.0               # host-side weight prescale (power of 2: exact)


def _build(repeat=1):
    nc = bacc.Bacc("TRN2", target_bir_lowering=False, debug=False,
                   num_devices=N_CORES)
    xT = nc.dram_tensor("xT", [D, T], FP8, kind="ExternalInput").ap()
    wqT = nc.dram_tensor("wqT", [H_LOC, D, E], FP8, kind="ExternalInput").ap()
    wkT = nc.dram_tensor("wkT", [H_LOC, D, E], FP8, kind="ExternalInput").ap()
    wvT = nc.dram_tensor("wvT", [H_LOC // 4, D, 4 * E], FP8,
                         kind="ExternalInput").ap()
    out = nc.dram_tensor("out", [H_LOC, E, T], F32, kind="ExternalOutput").ap()

    with tile.TileContext(nc) as tc:
        with (
            tc.tile_pool(name="xpool", bufs=1) as xpool,
            tc.tile_pool(name="wqk", bufs=2) as wqk,
            tc.tile_pool(name="wvp", bufs=2) as wvp,
            tc.tile_pool(name="qk", bufs=2) as qk,
            tc.tile_pool(name="vpool", bufs=2) as vpool,
            tc.tile_pool(name="ptp", bufs=2) as ptp,
            tc.tile_pool(name="outp", bufs=4) as outp,
            tc.tile_pool(name="smallp", bufs=2) as smallp,
            tc.tile_pool(name="onesp", bufs=1) as onesp,
            tc.tile_pool(name="stps", bufs=2, space="PSUM") as stps,
            tc.tile_pool(name="projps", bufs=2, space="PSUM") as projps,
            tc.tile_pool(name="ctps", bufs=1, space="PSUM") as ctps,
            tc.tile_pool(name="sumps", bufs=1, space="PSUM") as sumps,
        ):
            pools = dict(xpool=xpool, wqk=wqk, wvp=wvp, qk=qk, vpool=vpool,
                         ptp=ptp, outp=outp, smallp=smallp, onesp=onesp,
                         stps=stps, projps=projps, ctps=ctps, sumps=sumps)
            for _rep in range(repeat):
                _kernel_rep(tc, nc, pools, xT, wqT, wkT, wvT, out)
    nc.compile()
    return nc


def _kernel_rep(tc, nc, pools, xT, wqT, wkT, wvT, out):
    xpool = pools["xpool"]; wqk = pools["wqk"]; wvp = pools["wvp"]
    qk = pools["qk"]; vpool = pools["vpool"]; ptp = pools["ptp"]
    outp = pools["outp"]; smallp = pools["smallp"]; onesp = pools["onesp"]
    stps = pools["stps"]; projps = pools["projps"]
    ctps = pools["ctps"]; sumps = pools["sumps"]

    ones = onesp.tile([P, 2, P], FP8)
    nc.vector.memset(ones[:], 1.0)

    # ---- input DMAs: weights for head 0 first so the PE can start early ----
    x_sb = xpool.tile([P, DT, T], FP8, tag="x")
    xTr = xT.rearrange("(c p) t -> p c t", p=P)

    def _load_w(h):
        wq_sb = wqk.tile([P, DT, E], FP8, tag="wq")
        nc.sync.dma_start(wq_sb[:], wqT[h].rearrange("(c p) e -> p c e", p=P))
        wk_sb = wqk.tile([P, DT, E], FP8, tag="wk")
        nc.sync.dma_start(wk_sb[:], wkT[h].rearrange("(c p) e -> p c e", p=P))
        return wq_sb, wk_sb

    def _load_wv(quad):
        wv_sb = wvp.tile([P, DT, 4 * E], FP8, tag="wv")
        wvr = wvT[quad].rearrange("(c p) e -> p c e", p=P)
        for c4 in range(0, DT, 4):
            nc.sync.dma_start(wv_sb[:, c4:c4 + 4, :], wvr[:, c4:c4 + 4, :])
        return wv_sb

    wq0_sb = wqk.tile([P, DT, E], FP8, tag="wq")
    nc.sync.dma_start(wq0_sb[:], wqT[0].rearrange("(c p) e -> p c e", p=P))
    nc.sync.dma_start(x_sb[:, 0:2, :], xTr[:, 0:2, :])
    wk0_sb = wqk.tile([P, DT, E], FP8, tag="wk")
    nc.sync.dma_start(wk0_sb[:], wkT[0].rearrange("(c p) e -> p c e", p=P))
    wv0_sb = _load_wv(0)
    for c in range(2, DT, 2):
        nc.sync.dma_start(x_sb[:, c:c + 2, :], xTr[:, c:c + 2, :])
    w0 = (wq0_sb, wk0_sb)

    # ---- PE filler chains (projections), issued one at a time between
    #      attention pairs so the in-order PE stream stays busy ----

    def _qk_chain(w_sb, oT, nt):
        ps = projps.tile([P, QW], F32, tag="proj")
        for c in range(DP):
            nc.tensor.matmul(
                ps[:], lhsT=w_sb[:, 2 * c:2 * c + 2, :],
                rhs=x_sb[:, 2 * c:2 * c + 2, nt * QW:(nt + 1) * QW],
                start=(c == 0), stop=(c == DP - 1), perf_mode=DR)
        nc.vector.tensor_scalar_mul(oT[:, nt * QW:(nt + 1) * QW], ps[:],
                                    1.0 / WS)

    def _v_chain(wv_sb, v_sb, kt):
        ps = projps.tile([P, 4 * E], F32, tag="proj")
        for c in range(DP):
            nc.tensor.matmul(
                ps[:], lhsT=x_sb[:, 2 * c:2 * c + 2, kt * P:(kt + 1) * P],
                rhs=wv_sb[:, 2 * c:2 * c + 2, :],
                start=(c == 0), stop=(c == DP - 1), perf_mode=DR)
        nc.vector.tensor_scalar_mul(v_sb[:, kt, :], ps[:], 1.0 / WS)

    def _proj_qk_chains(h, w=None):
        """Returns (qT, kT, [chain thunks]) without issuing the chains."""
        wq_sb, wk_sb = w if w is not None else _load_w(h)
        qT = qk.tile([P, T], FP8, tag="qT")
        kT_sb = qk.tile([P, T], FP8, tag="kT")
        thunks = []
        for w_sb, oT in ((wq_sb, qT), (wk_sb, kT_sb)):
            for nt in range(QC):
                thunks.append(
                    lambda w_sb=w_sb, oT=oT, nt=nt: _qk_chain(w_sb, oT, nt))
        return qT, kT_sb, thunks

    def _proj_v_chains(quad, wv_sb):
        v_sb = vpool.tile([P, KT, 4 * E], FP8, tag="v")
        thunks = [lambda wv_sb=wv_sb, v_sb=v_sb, kt=kt:
                  _v_chain(wv_sb, v_sb, kt) for kt in range(KT)]
        return v_sb, thunks

    def _attn(h, hi, qT, kT_sb, v_sb, fillers):
        """Attention for head h; pops one filler thunk per pair iteration."""
        fi = 0
        for qc in range(QC):
            ct = ctps.tile([P, QW], F32, tag="ct")
            sm = sumps.tile([P, QW], F32, tag="sum")
            pt = ptp.tile([P, KT, QW], FP8, tag="pt")
            for i in range(KP):
                st = stps.tile([P, 2, QW], F32, tag="st")
                nc.tensor.matmul(
                    st[:, 0, :], lhsT=kT_sb[:, (2 * i) * P:(2 * i + 1) * P],
                    rhs=qT[:, qc * QW:(qc + 1) * QW], start=True, stop=True)
                nc.tensor.matmul(
                    st[:, 1, :], lhsT=kT_sb[:, (2 * i + 1) * P:(2 * i + 2) * P],
                    rhs=qT[:, qc * QW:(qc + 1) * QW], start=True, stop=True)
                nc.scalar.activation(
                    pt[:, 2 * i:2 * i + 2, :], st[:, :, :],
                    mybir.ActivationFunctionType.Exp,
                    scale=EXP_SCALE, bias=EXP_BIAS)
                nc.tensor.matmul(
                    ct[:], lhsT=v_sb[:, 2 * i:2 * i + 2, hi * E:(hi + 1) * E],
                    rhs=pt[:, 2 * i:2 * i + 2, :],
                    start=(i == 0), stop=(i == KP - 1), perf_mode=DR)
                nc.tensor.matmul(
                    sm[:], lhsT=ones[:], rhs=pt[:, 2 * i:2 * i + 2, :],
                    start=(i == 0), stop=(i == KP - 1), perf_mode=DR)
                # keep the PE busy while ACT works on the next exp
                if fi < len(fillers) and (i % 2 == 1):
                    fillers[fi](); fi += 1
            rec = smallp.tile([P, QW], F32, tag="rec")
            nc.vector.reciprocal(rec[:], sm[:])
            ot = outp.tile([P, QW], F32, tag="ot")
            nc.vector.tensor_mul(ot[:], ct[:], rec[:])
            nc.sync.dma_start(out[h, :, qc * QW:(qc + 1) * QW], ot[:])
        # drain any remaining fillers at head end
        while fi < len(fillers):
            fillers[fi](); fi += 1

    # ---- schedule: startup head0 QK + quad0 V serially, then attention
    #      heads with next-head projections interleaved ----
    qT0, kT0, qk_thunks = _proj_qk_chains(0, w=w0)
    v_sb0, v_thunks = _proj_v_chains(0, wv0_sb)
    for th in qk_thunks:
        th()
    for th in v_thunks:
        th()

    cur_qk = (qT0, kT0)
    cur_v = v_sb0
    for h in range(H_LOC):
        quad, hi = divmod(h, 4)
        fillers = []
        next_qk = None
        next_v = None
        if h + 1 < H_LOC:
            nqT, nkT, nthunks = _proj_qk_chains(h + 1)
            fillers.extend(nthunks)
            next_qk = (nqT, nkT)
        if hi == 3 and quad + 1 < H_LOC // 4:
            wv_sb = _load_wv(quad + 1)
            nv_sb, nvthunks = _proj_v_chains(quad + 1, wv_sb)
            fillers.extend(nvthunks)
            next_v = nv_sb
        if hi == 2 and quad + 1 < H_LOC // 4:
            pass  # wv DMA could start here; kept simple
        _attn(h, hi, cur_qk[0], cur_qk[1], cur_v, fillers)
        if next_qk is not None:
            cur_qk = next_qk
        if next_v is not None:
            cur_v = next_v


_NC_CACHE = {}


def _get_nc():
    if "nc" not in _NC_CACHE:
        _NC_CACHE["nc"] = _build()
    return _NC_CACHE["nc"]


def _prep_in_maps(x, Wq, Wk, Wv):
    f8 = ml_dtypes.float8_e4m3
    x8 = np.asarray(x, dtype=np.float32).astype(f8)
    Wq8 = (np.asarray(Wq, dtype=np.float32) * WS).astype(f8)
    Wk8 = (np.asarray(Wk, dtype=np.float32) * WS).astype(f8)
    Wv8 = (np.asarray(Wv, dtype=np.float32) * WS).astype(f8)

    xT_by_b = [np.ascontiguousarray(x8[b].T) for b in range(B)]
    wq_by_g, wk_by_g, wv_by_g = [], [], []
    for g in range(2):
        sl = slice(g * H_LOC * E, (g + 1) * H_LOC * E)
        wq_by_g.append(np.ascontiguousarray(
            Wq8[sl].reshape(H_LOC, E, D).transpose(0, 2, 1)))
        wk_by_g.append(np.ascontiguousarray(
            Wk8[sl].reshape(H_LOC, E, D).transpose(0, 2, 1)))
        wv_by_g.append(np.ascontiguousarray(
            Wv8[sl].reshape(H_LOC // 4, 4, E, D)
            .transpose(0, 3, 1, 2).reshape(H_LOC // 4, D, 4 * E)))

    in_maps = []
    for c in range(N_CORES):
        b, g = divmod(c, 2)
        in_maps.append({
            "xT": xT_by_b[b],
            "wqT": wq_by_g[g],
            "wkT": wk_by_g[g],
            "wvT": wv_by_g[g],
        })
    return in_maps


def run_sharded(x, Wq, Wk, Wv, **spmd_kwargs):
    """Build+run on 8 cores; returns (full_output, BassKernelResults)."""
    nc = _get_nc()
    in_maps = _prep_in_maps(x, Wq, Wk, Wv)
    res = run_bass_kernel_spmd(nc, in_maps, list(range(N_CORES)), **spmd_kwargs)
    full = np.empty((B, H, T, E), np.float32)
    for c in range(N_CORES):
        b, g = divmod(c, 2)
        oc = res.results[c]["out"]  # [H_LOC, E, T]
        full[b, g * H_LOC:(g + 1) * H_LOC] = oc.transpose(0, 2, 1)
    return full, res


def kernel(x, Wq, Wk, Wv):
    full, _ = run_sharded(x, Wq, Wk, Wv)
    return full


# revision 3
# speedup vs baseline: 128.1179x; 128.1179x over previous
"""Multi-head attention (B=4, T=2048, D=2048, H=16, E=128) on 8 TRN2 NeuronCores.

Sharding: batch (4) x head-group (2 groups of 8 heads) -> 8 cores.
Per core: q/k/v projections for its 8 heads + softmax(QK^T/sqrt(E))V.

FP8 (e4m3) design:
  - all matmul operands are fp8e4; accumulation stays fp32 in PSUM.
  - projections + context + softmax-denominator use DoubleRow perf mode
    (two 128-deep contraction chunks per instruction, 0.5 cycles/row).
  - weights are host-prescaled by 64 so their sigma~1.4 avoids the fp8
    denormal range; the PSUM->SBUF copy rescales by 1/64 (DVE tensor_scalar).
  - scores S^T[k,q] = K Q^T per 128-key tile, plain fp8 matmul.
  - softmax: exp((s - 3*sqrt(E))/sqrt(E)) on ACT over [128,1024] tile pairs
    (the -3 shift cancels between numerator and denominator and keeps
    exp() outputs ~15 max, far below fp8e4's 240 ceiling).
  - denominator: DoubleRow matmul with an all-ones lhsT accumulating the
    fp8 P^T pair tiles over keys; replicated over all 128 partitions so the
    final normalize is a plain elementwise multiply.
  - output written as C^T [h,E,T] fp32; host transposes back to [h,T,E].

PE order is hand-interleaved: the next head's Q/K projection chains (and the
next quad's V chains) are issued between attention pair iterations so the
in-order PE queue fills the stalls where PE waits on ACT's exp.
"""

import math
import sys

sys.path.insert(0, "/opt/trn_rl_repo")

import ml_dtypes
import numpy as np

import concourse.bass as bass  # noqa: F401  (registers engine methods)
import concourse.mybir as mybir
import concourse.tile as tile
from concourse import bacc
from concourse.bass_utils import run_bass_kernel_spmd

B, T, D, H, E = 4, 2048, 2048, 16, 128
N_CORES = 8
H_LOC = H // 2          # heads per core
P = 128                 # partitions
DT = D // P             # contraction chunks for projections
DP = DT // 2            # DoubleRow pairs over D
KT = T // P             # key tiles
KP = KT // 2            # DoubleRow pairs over keys
QW = 512                # q-chunk width (one PSUM bank of fp32)
QC = T // QW
FP8 = mybir.dt.float8e4
F32 = mybir.dt.float32
DR = mybir.MatmulPerfMode.DoubleRow
EXP_SCALE = 1.0 / math.sqrt(E)
EXP_BIAS = -3.0         # softmax shift; cancels in the normalize
WS = 64.0               # host-side weight prescale (power of 2: exact)


def _build(repeat=1):
    nc = bacc.Bacc("TRN2", target_bir_lowering=False, debug=False,
                   num_devices=N_CORES)
    xT = nc.dram_tensor("xT", [D, T], FP8, kind="ExternalInput").ap()
    wqT = nc.dram_tensor("wqT", [H_LOC, D, E], FP8, kind="ExternalInput").ap()
    wkT = nc.dram_tensor("wkT", [H_LOC, D, E], FP8, kind="ExternalInput").ap()
    wvT = nc.dram_tensor("wvT", [H_LOC // 4, D, 4 * E], FP8,
                         kind="ExternalInput").ap()
    out = nc.dram_tensor("out", [H_LOC, E, T], F32, kind="ExternalOutput").ap()

    with tile.TileContext(nc) as tc:
        with (
            tc.tile_pool(name="xpool", bufs=1) as xpool,
            tc.tile_pool(name="wqk", bufs=2) as wqk,
            tc.tile_pool(name="wvp", bufs=2) as wvp,
            tc.tile_pool(name="qk", bufs=2) as qk,
            tc.tile_pool(name="vpool", bufs=2) as vpool,
            tc.tile_pool(name="ptp", bufs=2) as ptp,
            tc.tile_pool(name="outp", bufs=4) as outp,
            tc.tile_pool(name="smallp", bufs=2) as smallp,
            tc.tile_pool(name="onesp", bufs=1) as onesp,
            tc.tile_pool(name="stps", bufs=2, space="PSUM") as stps,
            tc.tile_pool(name="projps", bufs=2, space="PSUM") as projps,
            tc.tile_pool(name="ctps", bufs=1, space="PSUM") as ctps,
            tc.tile_pool(name="sumps", bufs=1, space="PSUM") as sumps,
        ):
            pools = dict(xpool=xpool, wqk=wqk, wvp=wvp, qk=qk, vpool=vpool,
                         ptp=ptp, outp=outp, smallp=smallp, onesp=onesp,
                         stps=stps, projps=projps, ctps=ctps, sumps=sumps)
            for _rep in range(repeat):
                _kernel_rep(tc, nc, pools, xT, wqT, wkT, wvT, out)
    nc.compile()
    return nc


def _kernel_rep(tc, nc, pools, xT, wqT, wkT, wvT, out):
    xpool = pools["xpool"]; wqk = pools["wqk"]; wvp = pools["wvp"]
    qk = pools["qk"]; vpool = pools["vpool"]; ptp = pools["ptp"]
    outp = pools["outp"]; smallp = pools["smallp"]; onesp = pools["onesp"]
    stps = pools["stps"]; projps = pools["projps"]
    ctps = pools["ctps"]; sumps = pools["sumps"]

    ones = onesp.tile([P, 2, P], FP8)
    nc.vector.memset(ones[:], 1.0)

    # ---- input DMAs: weights for head 0 first so the PE can start early ----
    x_sb = xpool.tile([P, DT, T], FP8, tag="x")
    xTr = xT.rearrange("(c p) t -> p c t", p=P)

    def _load_w(h):
        wq_sb = wqk.tile([P, DT, E], FP8, tag="wq")
        nc.sync.dma_start(wq_sb[:], wqT[h].rearrange("(c p) e -> p c e", p=P))
        wk_sb = wqk.tile([P, DT, E], FP8, tag="wk")
        nc.sync.dma_start(wk_sb[:], wkT[h].rearrange("(c p) e -> p c e", p=P))
        return wq_sb, wk_sb

    def _load_wv(quad):
        wv_sb = wvp.tile([P, DT, 4 * E], FP8, tag="wv")
        wvr = wvT[quad].rearrange("(c p) e -> p c e", p=P)
        for c4 in range(0, DT, 4):
            nc.sync.dma_start(wv_sb[:, c4:c4 + 4, :], wvr[:, c4:c4 + 4, :])
        return wv_sb

    wq0_sb = wqk.tile([P, DT, E], FP8, tag="wq")
    nc.sync.dma_start(wq0_sb[:], wqT[0].rearrange("(c p) e -> p c e", p=P))
    nc.sync.dma_start(x_sb[:, 0:2, :], xTr[:, 0:2, :])
    wk0_sb = wqk.tile([P, DT, E], FP8, tag="wk")
    nc.sync.dma_start(wk0_sb[:], wkT[0].rearrange("(c p) e -> p c e", p=P))
    wv0_sb = _load_wv(0)
    for c in range(2, DT, 2):
        nc.sync.dma_start(x_sb[:, c:c + 2, :], xTr[:, c:c + 2, :])
    w0 = (wq0_sb, wk0_sb)

    # ---- PE filler chains (projections), issued one at a time between
    #      attention pairs so the in-order PE stream stays busy ----

    def _qk_chain(w_sb, oT, nt):
        ps = projps.tile([P, QW], F32, tag="proj")
        for c in range(DP):
            nc.tensor.matmul(
                ps[:], lhsT=w_sb[:, 2 * c:2 * c + 2, :],
                rhs=x_sb[:, 2 * c:2 * c + 2, nt * QW:(nt + 1) * QW],
                start=(c == 0), stop=(c == DP - 1), perf_mode=DR)
        nc.vector.tensor_scalar_mul(oT[:, nt * QW:(nt + 1) * QW], ps[:],
                                    1.0 / WS)

    def _v_chain(wv_sb, v_sb, kt):
        ps = projps.tile([P, 4 * E], F32, tag="proj")
        for c in range(DP):
            nc.tensor.matmul(
                ps[:], lhsT=x_sb[:, 2 * c:2 * c + 2, kt * P:(kt + 1) * P],
                rhs=wv_sb[:, 2 * c:2 * c + 2, :],
                start=(c == 0), stop=(c == DP - 1), perf_mode=DR)
        nc.vector.tensor_scalar_mul(v_sb[:, kt, :], ps[:], 1.0 / WS)

    def _proj_qk_chains(h, w=None):
        """Returns (qT, kT, [chain thunks]) without issuing the chains."""
        wq_sb, wk_sb = w if w is not None else _load_w(h)
        qT = qk.tile([P, T], FP8, tag="qT")
        kT_sb = qk.tile([P, T], FP8, tag="kT")
        thunks = []
        for w_sb, oT in ((wq_sb, qT), (wk_sb, kT_sb)):
            for nt in range(QC):
                thunks.append(
                    lambda w_sb=w_sb, oT=oT, nt=nt: _qk_chain(w_sb, oT, nt))
        return qT, kT_sb, thunks

    def _proj_v_chains(quad, wv_sb):
        v_sb = vpool.tile([P, KT, 4 * E], FP8, tag="v")
        thunks = [lambda wv_sb=wv_sb, v_sb=v_sb, kt=kt:
                  _v_chain(wv_sb, v_sb, kt) for kt in range(KT)]
        return v_sb, thunks

    def _attn(h, hi, qT, kT_sb, v_sb, fillers):
        """Attention for head h; pops one filler thunk per pair iteration."""
        fi = 0
        for qc in range(QC):
            ct = ctps.tile([P, QW], F32, tag="ct")
            sm = sumps.tile([P, QW], F32, tag="sum")
            pt = ptp.tile([P, KT, QW], FP8, tag="pt")
            for i in range(KP):
                st = stps.tile([P, 2, QW], F32, tag="st")
                nc.tensor.matmul(
                    st[:, 0, :], lhsT=kT_sb[:, (2 * i) * P:(2 * i + 1) * P],
                    rhs=qT[:, qc * QW:(qc + 1) * QW], start=True, stop=True)
                nc.tensor.matmul(
                    st[:, 1, :], lhsT=kT_sb[:, (2 * i + 1) * P:(2 * i + 2) * P],
                    rhs=qT[:, qc * QW:(qc + 1) * QW], start=True, stop=True)
                nc.scalar.activation(
                    pt[:, 2 * i:2 * i + 2, :], st[:, :, :],
                    mybir.ActivationFunctionType.Exp,
                    scale=EXP_SCALE, bias=EXP_BIAS)
                nc.tensor.matmul(
                    ct[:], lhsT=v_sb[:, 2 * i:2 * i + 2, hi * E:(hi + 1) * E],
                    rhs=pt[:, 2 * i:2 * i + 2, :],
                    start=(i == 0), stop=(i == KP - 1), perf_mode=DR)
                nc.tensor.matmul(
                    sm[:], lhsT=ones[:], rhs=pt[:, 2 * i:2 * i + 2, :],
                    start=(i == 0), stop=(i == KP - 1), perf_mode=DR)
                # keep the PE busy while ACT works on the next exp
                if fi < len(fillers) and (i % 2 == 1):
                    fillers[fi](); fi += 1
            rec = smallp.tile([P, QW], F32, tag="rec")
            nc.vector.reciprocal(rec[:], sm[:])
            ot = outp.tile([P, QW], F32, tag="ot")
            nc.vector.tensor_mul(ot[:], ct[:], rec[:])
            nc.sync.dma_start(out[h, :, qc * QW:(qc + 1) * QW], ot[:])
        # drain any remaining fillers at head end
        while fi < len(fillers):
            fillers[fi](); fi += 1

    # ---- schedule: startup head0 QK + quad0 V serially, then attention
    #      heads with next-head projections interleaved ----
    qT0, kT0, qk_thunks = _proj_qk_chains(0, w=w0)
    v_sb0, v_thunks = _proj_v_chains(0, wv0_sb)
    for th in qk_thunks:
        th()
    for th in v_thunks:
        th()

    cur_qk = (qT0, kT0)
    cur_v = v_sb0
    pending_wv = None
    for h in range(H_LOC):
        quad, hi = divmod(h, 4)
        fillers = []
        next_qk = None
        next_v = None
        if h + 1 < H_LOC:
            nqT, nkT, nthunks = _proj_qk_chains(h + 1)
            fillers.extend(nthunks)
            next_qk = (nqT, nkT)
        if hi == 2 and quad + 1 < H_LOC // 4:
            pending_wv = _load_wv(quad + 1)
        if hi == 3 and quad + 1 < H_LOC // 4:
            nv_sb, nvthunks = _proj_v_chains(quad + 1, pending_wv)
            fillers.extend(nvthunks)
            next_v = nv_sb
        _attn(h, hi, cur_qk[0], cur_qk[1], cur_v, fillers)
        if next_qk is not None:
            cur_qk = next_qk
        if next_v is not None:
            cur_v = next_v


_NC_CACHE = {}


def _get_nc():
    if "nc" not in _NC_CACHE:
        _NC_CACHE["nc"] = _build()
    return _NC_CACHE["nc"]


def _prep_in_maps(x, Wq, Wk, Wv):
    f8 = ml_dtypes.float8_e4m3
    x8 = np.asarray(x, dtype=np.float32).astype(f8)
    Wq8 = (np.asarray(Wq, dtype=np.float32) * WS).astype(f8)
    Wk8 = (np.asarray(Wk, dtype=np.float32) * WS).astype(f8)
    Wv8 = (np.asarray(Wv, dtype=np.float32) * WS).astype(f8)

    xT_by_b = [np.ascontiguousarray(x8[b].T) for b in range(B)]
    wq_by_g, wk_by_g, wv_by_g = [], [], []
    for g in range(2):
        sl = slice(g * H_LOC * E, (g + 1) * H_LOC * E)
        wq_by_g.append(np.ascontiguousarray(
            Wq8[sl].reshape(H_LOC, E, D).transpose(0, 2, 1)))
        wk_by_g.append(np.ascontiguousarray(
            Wk8[sl].reshape(H_LOC, E, D).transpose(0, 2, 1)))
        wv_by_g.append(np.ascontiguousarray(
            Wv8[sl].reshape(H_LOC // 4, 4, E, D)
            .transpose(0, 3, 1, 2).reshape(H_LOC // 4, D, 4 * E)))

    in_maps = []
    for c in range(N_CORES):
        b, g = divmod(c, 2)
        in_maps.append({
            "xT": xT_by_b[b],
            "wqT": wq_by_g[g],
            "wkT": wk_by_g[g],
            "wvT": wv_by_g[g],
        })
    return in_maps


def run_sharded(x, Wq, Wk, Wv, **spmd_kwargs):
    """Build+run on 8 cores; returns (full_output, BassKernelResults)."""
    nc = _get_nc()
    in_maps = _prep_in_maps(x, Wq, Wk, Wv)
    res = run_bass_kernel_spmd(nc, in_maps, list(range(N_CORES)), **spmd_kwargs)
    full = np.empty((B, H, T, E), np.float32)
    for c in range(N_CORES):
        b, g = divmod(c, 2)
        oc = res.results[c]["out"]  # [H_LOC, E, T]
        full[b, g * H_LOC:(g + 1) * H_LOC] = oc.transpose(0, 2, 1)
    return full, res


def kernel(x, Wq, Wk, Wv):
    full, _ = run_sharded(x, Wq, Wk, Wv)
    return full


# revision 6
# speedup vs baseline: 182.6161x; 1.4254x over previous
"""Multi-head attention (B=4, T=2048, D=2048, H=16, E=128) on 8 TRN2 NeuronCores.

Sharding: batch (4) x head-group (2 groups of 8 heads) -> 8 cores.
Per core: q/k/v projections for its 8 heads + softmax(QK^T/sqrt(E))V.

All-bf16 layout (fp8 fails the 2e-2 accuracy gate: the softmax is highly
concentrated, so quantization noise does not average out):
  - host passes x^T [D,T] and per-head W^T [D,E] (bf16) so the contraction
    dim D lands on SBUF partitions directly.
  - Q^T,K^T computed as [E,T] (lhsT=W^T chunk, rhs=x^T chunk).
  - V computed as [T,E] (lhsT=x^T chunk, rhs=Wv^T chunk), head-quads at N=512.
  - scores computed transposed: S^T[k,q] = (K Q^T) into [128,2,512] PSUM
    pairs so softmax-exp runs as [128,1024] ACT instructions (the +352-cycle
    per-instruction ACT overhead was the baseline's hidden critical path).
  - P^T pair tiles feed C^T[e,q] = V^T P^T.
  - softmax denominators: DVE tree-sums the 8 P^T pair-tiles of a q-chunk
    elementwise (bf16, [128,1024]-wide adds), one all-ones matmul per
    q-chunk reduces over partitions into PSUM.
  - the next head's Q/K projection chains (and next quad's V chains) are
    issued between attention pair iterations, so the in-order PE queue
    fills the stalls where PE would wait on ACT's exp.
  - output written as C^T [h,E,T]; host transposes back to [h,T,E].
"""

import math
import sys

sys.path.insert(0, "/opt/trn_rl_repo")

import ml_dtypes
import numpy as np

import concourse.bass as bass  # noqa: F401  (registers engine methods)
import concourse.mybir as mybir
import concourse.tile as tile
from concourse import bacc
from concourse.bass_utils import run_bass_kernel_spmd

B, T, D, H, E = 4, 2048, 2048, 16, 128
N_CORES = 8
H_LOC = H // 2          # heads per core
P = 128                 # partitions
DT = D // P             # contraction chunks for projections
KT = T // P             # key tiles
KP = KT // 2            # key tile pairs
QW = 512                # q-chunk width (one PSUM bank of fp32)
QC = T // QW
BF16 = mybir.dt.bfloat16
F32 = mybir.dt.float32
EXP_SCALE = 1.0 / math.sqrt(E)


def _build(repeat=1):
    nc = bacc.Bacc("TRN2", target_bir_lowering=False, debug=False,
                   num_devices=N_CORES)
    xT = nc.dram_tensor("xT", [D, T], BF16, kind="ExternalInput").ap()
    wqT = nc.dram_tensor("wqT", [H_LOC, D, E], BF16, kind="ExternalInput").ap()
    wkT = nc.dram_tensor("wkT", [H_LOC, D, E], BF16, kind="ExternalInput").ap()
    wvT = nc.dram_tensor("wvT", [H_LOC // 4, D, 4 * E], BF16,
                         kind="ExternalInput").ap()
    out = nc.dram_tensor("out", [H_LOC, E, T], F32, kind="ExternalOutput").ap()

    with tile.TileContext(nc) as tc:
        with (
            tc.tile_pool(name="xpool", bufs=1) as xpool,
            tc.tile_pool(name="wqk", bufs=2) as wqk,
            tc.tile_pool(name="wvp", bufs=1) as wvp,
            tc.tile_pool(name="qk", bufs=2) as qk,
            tc.tile_pool(name="vpool", bufs=2) as vpool,
            tc.tile_pool(name="ptp", bufs=2) as ptp,
            tc.tile_pool(name="outp", bufs=2) as outp,
            tc.tile_pool(name="smallp", bufs=2) as smallp,
            tc.tile_pool(name="dsum", bufs=1) as dsum,
            tc.tile_pool(name="onesp", bufs=1) as onesp,
            tc.tile_pool(name="stps", bufs=2, space="PSUM") as stps,
            tc.tile_pool(name="projps", bufs=2, space="PSUM") as projps,
            tc.tile_pool(name="ctps", bufs=1, space="PSUM") as ctps,
            tc.tile_pool(name="sumps", bufs=1, space="PSUM") as sumps,
        ):
            pools = dict(xpool=xpool, wqk=wqk, wvp=wvp, qk=qk, vpool=vpool,
                         ptp=ptp, outp=outp, smallp=smallp, dsum=dsum,
                         onesp=onesp, stps=stps, projps=projps, ctps=ctps,
                         sumps=sumps)
            for _rep in range(repeat):
                _kernel_rep(tc, nc, pools, xT, wqT, wkT, wvT, out)
    nc.compile()
    return nc


def _kernel_rep(tc, nc, pools, xT, wqT, wkT, wvT, out):
    xpool = pools["xpool"]; wqk = pools["wqk"]; wvp = pools["wvp"]
    qk = pools["qk"]; vpool = pools["vpool"]; ptp = pools["ptp"]
    outp = pools["outp"]; smallp = pools["smallp"]; dsum = pools["dsum"]
    onesp = pools["onesp"]; stps = pools["stps"]; projps = pools["projps"]
    ctps = pools["ctps"]; sumps = pools["sumps"]

    ones = onesp.tile([P, P], BF16)
    nc.vector.memset(ones[:], 1.0)

    # ---- input DMAs: weights for head 0 first so the PE can start early ----
    xTr = xT.rearrange("(c p) t -> p c t", p=P)
    xs = []
    for c in range(DT):
        xt = xpool.tile([P, T], BF16, tag=f"x{c}")
        xs.append(xt)

    def _load_w(h):
        wq_sb = wqk.tile([P, DT, E], BF16, tag="wq")
        nc.sync.dma_start(wq_sb[:], wqT[h].rearrange("(c p) e -> p c e", p=P))
        wk_sb = wqk.tile([P, DT, E], BF16, tag="wk")
        nc.sync.dma_start(wk_sb[:], wkT[h].rearrange("(c p) e -> p c e", p=P))
        return wq_sb, wk_sb

    def _load_wv(quad):
        wv_sb = wvp.tile([P, DT, 4 * E], BF16, tag="wv")
        wvr = wvT[quad].rearrange("(c p) e -> p c e", p=P)
        for c4 in range(0, DT, 4):
            nc.sync.dma_start(wv_sb[:, c4:c4 + 4, :], wvr[:, c4:c4 + 4, :])
        return wv_sb

    wq0_sb = wqk.tile([P, DT, E], BF16, tag="wq")
    nc.sync.dma_start(wq0_sb[:], wqT[0].rearrange("(c p) e -> p c e", p=P))
    nc.sync.dma_start(xs[0][:], xTr[:, 0, :])
    wk0_sb = wqk.tile([P, DT, E], BF16, tag="wk")
    nc.sync.dma_start(wk0_sb[:], wkT[0].rearrange("(c p) e -> p c e", p=P))
    wv0_sb = _load_wv(0)
    for c in range(1, DT):
        nc.sync.dma_start(xs[c][:], xTr[:, c, :])
    w0 = (wq0_sb, wk0_sb)

    # ---- projection chains, issued one at a time between attention pairs ----

    def _qk_chain(w_sb, oT, nt, pool=None, tag="proj"):
        pool = pool or projps
        ps = pool.tile([P, QW], F32, tag=tag)
        for c in range(DT):
            nc.tensor.matmul(
                ps[:], lhsT=w_sb[:, c, :],
                rhs=xs[c][:, nt * QW:(nt + 1) * QW],
                start=(c == 0), stop=(c == DT - 1))
        nc.vector.tensor_copy(oT[:, nt * QW:(nt + 1) * QW], ps[:])

    def _v_chain(wv_sb, v_sb, kt, pool=None, tag="proj"):
        pool = pool or projps
        ps = pool.tile([P, 4 * E], F32, tag=tag)
        for c in range(DT):
            nc.tensor.matmul(
                ps[:], lhsT=xs[c][:, kt * P:(kt + 1) * P],
                rhs=wv_sb[:, c, :],
                start=(c == 0), stop=(c == DT - 1))
        nc.vector.tensor_copy(v_sb[:, kt, :], ps[:])

    def _proj_qk_chains(h, w=None):
        wq_sb, wk_sb = w if w is not None else _load_w(h)
        qT = qk.tile([P, T], BF16, tag="qT")
        kT_sb = qk.tile([P, T], BF16, tag="kT")
        thunks = []
        for w_sb, oT in ((wq_sb, qT), (wk_sb, kT_sb)):
            for nt in range(QC):
                thunks.append(
                    lambda w_sb=w_sb, oT=oT, nt=nt: _qk_chain(w_sb, oT, nt))
        return qT, kT_sb, thunks

    def _proj_v_chains(quad, wv_sb):
        v_sb = vpool.tile([P, KT, 4 * E], BF16, tag="v")
        thunks = [lambda wv_sb=wv_sb, v_sb=v_sb, kt=kt:
                  _v_chain(wv_sb, v_sb, kt) for kt in range(KT)]
        return v_sb, thunks

    def _attn(h, hi, qT, kT_sb, v_sb, fillers):
        """Attention for head h; pops filler thunks between pair iters."""
        fi = 0
        for qc in range(QC):
            ct = ctps.tile([P, QW], F32, tag="ct")
            sm = sumps.tile([P, QW], F32, tag="sum")
            pt = ptp.tile([P, KT, QW], BF16, tag="pt")
            d4 = [None] * 4
            st_prev = [None]

            def _ctx(i, ct=ct, pt=pt, v_sb=v_sb, hi=hi):
                nc.tensor.matmul(
                    ct[:], lhsT=v_sb[:, 2 * i, hi * E:(hi + 1) * E],
                    rhs=pt[:, 2 * i, :], start=(i == 0), stop=False)
                nc.tensor.matmul(
                    ct[:], lhsT=v_sb[:, 2 * i + 1, hi * E:(hi + 1) * E],
                    rhs=pt[:, 2 * i + 1, :], start=False, stop=(i == KP - 1))

            ds = [None]
            for i in range(KP):
                st = stps.tile([P, 2, QW], F32, tag="st")
                nc.tensor.matmul(
                    st[:, 0, :], lhsT=kT_sb[:, (2 * i) * P:(2 * i + 1) * P],
                    rhs=qT[:, qc * QW:(qc + 1) * QW], start=True, stop=True)
                nc.tensor.matmul(
                    st[:, 1, :], lhsT=kT_sb[:, (2 * i + 1) * P:(2 * i + 2) * P],
                    rhs=qT[:, qc * QW:(qc + 1) * QW], start=True, stop=True)
                nc.scalar.activation(
                    pt[:, 2 * i:2 * i + 2, :], st[:, :, :],
                    mybir.ActivationFunctionType.Exp, scale=EXP_SCALE)
                # context runs two pairs behind exp so PE never waits on ACT
                if i >= 2:
                    _ctx(i - 2)
                if fi < len(fillers) and (i % 2 == 1):
                    fillers[fi](); fi += 1
                # DVE pair-tree for the denominator
                if i % 2 == 1:
                    dd = dsum.tile([P, 2, QW], BF16, tag=f"d4_{i // 2}")
                    nc.vector.tensor_add(dd[:], pt[:, 2 * i - 2:2 * i, :],
                                         pt[:, 2 * i:2 * i + 2, :])
                    d4[i // 2] = dd
                    if i == 3:
                        d8a = dsum.tile([P, 2, QW], BF16, tag="d8a")
                        nc.vector.tensor_add(d8a[:], d4[0][:], d4[1][:])
                        d4[0] = d8a
                    if i == 7:
                        d8b = dsum.tile([P, 2, QW], BF16, tag="d8b")
                        nc.vector.tensor_add(d8b[:], d4[2][:], d4[3][:])
                        d16 = dsum.tile([P, 2, QW], BF16, tag="d16")
                        nc.vector.tensor_add(d16[:], d4[0][:], d8b[:])
                        dst = dsum.tile([P, QW], BF16, tag="ds")
                        nc.vector.tensor_add(dst[:], d16[:, 0, :],
                                             d16[:, 1, :])
                        ds[0] = dst
            _ctx(KP - 2)
            _ctx(KP - 1)
            # denominator reduce after the last ctx: the DVE tree has long
            # finished, so this never stalls the PE
            nc.tensor.matmul(sm[:], lhsT=ones[:], rhs=ds[0][:],
                             start=True, stop=True)
            del st_prev
            rec = smallp.tile([P, QW], F32, tag="rec")
            nc.vector.reciprocal(rec[:], sm[:])
            ot = outp.tile([P, QW], F32, tag="ot")
            nc.vector.tensor_mul(ot[:], ct[:], rec[:])
            nc.sync.dma_start(out[h, :, qc * QW:(qc + 1) * QW], ot[:])
        while fi < len(fillers):
            fillers[fi](); fi += 1

    # ---- startup: head0 QK + quad0 V, borrowing idle attention banks for
    #      extra concurrent chains while x streams in ----
    qT0, kT0, qk_thunks = _proj_qk_chains(0, w=w0)
    v_sb0, v_thunks = _proj_v_chains(0, wv0_sb)
    extra = [(ctps, "ct"), (sumps, "sum"), (stps, "st")]
    for idx, th in enumerate(qk_thunks):
        if idx % 2 == 1 and idx // 2 < len(extra):
            pool, tag = extra[idx // 2]
            w_sb, oT, nt = th.__defaults__
            _qk_chain(w_sb, oT, nt, pool=pool, tag=tag)
        else:
            th()
    for th in v_thunks:
        th()

    cur_qk = (qT0, kT0)
    cur_v = v_sb0
    pending_wv = None
    for h in range(H_LOC):
        quad, hi = divmod(h, 4)
        fillers = []
        next_qk = None
        next_v = None
        if h + 1 < H_LOC:
            nqT, nkT, nthunks = _proj_qk_chains(h + 1)
            fillers.extend(nthunks)
            next_qk = (nqT, nkT)
        if hi == 2 and quad + 1 < H_LOC // 4:
            pending_wv = _load_wv(quad + 1)
        if hi == 3 and quad + 1 < H_LOC // 4:
            nv_sb, nvthunks = _proj_v_chains(quad + 1, pending_wv)
            fillers.extend(nvthunks)
            next_v = nv_sb
        _attn(h, hi, cur_qk[0], cur_qk[1], cur_v, fillers)
        if next_qk is not None:
            cur_qk = next_qk
        if next_v is not None:
            cur_v = next_v


_NC_CACHE = {}


def _get_nc():
    if "nc" not in _NC_CACHE:
        _NC_CACHE["nc"] = _build()
    return _NC_CACHE["nc"]


def _prep_in_maps(x, Wq, Wk, Wv):
    bf = ml_dtypes.bfloat16
    x16 = np.asarray(x).astype(bf)
    Wq16 = np.asarray(Wq).astype(bf)
    Wk16 = np.asarray(Wk).astype(bf)
    Wv16 = np.asarray(Wv).astype(bf)

    xT_by_b = [np.ascontiguousarray(x16[b].T) for b in range(B)]
    wq_by_g, wk_by_g, wv_by_g = [], [], []
    for g in range(2):
        sl = slice(g * H_LOC * E, (g + 1) * H_LOC * E)
        wq_by_g.append(np.ascontiguousarray(
            Wq16[sl].reshape(H_LOC, E, D).transpose(0, 2, 1)))
        wk_by_g.append(np.ascontiguousarray(
            Wk16[sl].reshape(H_LOC, E, D).transpose(0, 2, 1)))
        wv_by_g.append(np.ascontiguousarray(
            Wv16[sl].reshape(H_LOC // 4, 4, E, D)
            .transpose(0, 3, 1, 2).reshape(H_LOC // 4, D, 4 * E)))

    in_maps = []
    for c in range(N_CORES):
        b, g = divmod(c, 2)
        in_maps.append({
            "xT": xT_by_b[b],
            "wqT": wq_by_g[g],
            "wkT": wk_by_g[g],
            "wvT": wv_by_g[g],
        })
    return in_maps


def run_sharded(x, Wq, Wk, Wv, **spmd_kwargs):
    """Build+run on 8 cores; returns (full_output, BassKernelResults)."""
    nc = _get_nc()
    in_maps = _prep_in_maps(x, Wq, Wk, Wv)
    res = run_bass_kernel_spmd(nc, in_maps, list(range(N_CORES)), **spmd_kwargs)
    full = np.empty((B, H, T, E), np.float32)
    for c in range(N_CORES):
        b, g = divmod(c, 2)
        oc = res.results[c]["out"]  # [H_LOC, E, T]
        full[b, g * H_LOC:(g + 1) * H_LOC] = oc.transpose(0, 2, 1)
    return full, res


def kernel(x, Wq, Wk, Wv):
    full, _ = run_sharded(x, Wq, Wk, Wv)
    return full
